# revision 1
# baseline (speedup 1.0000x reference)
"""CenterLoss update kernel for Trainium2, 8-core SPMD — class-sharded.

Reference computation (N=16384 samples, C=10000 classes, D=128 dims):
    embeded_labels = labels @ center          # [N,D] gather via one-hot
    diff = embeded_labels - embeded_preds
    grad = (labels.T @ diff) / (counts + 1)   # counts = labels.T @ ones
    out  = center - 0.5 * grad

Because each row of ``labels`` is one-hot, ``labels.T @ labels == diag(counts)``,
so the whole thing collapses to a single pass over ``labels``:

    S      = labels.T @ embeded_preds         # [C,D] per-class sum of preds
    counts = column sums of labels            # [C]
    out    = beta * center + gamma * S
             beta  = 1 - 0.5*counts/(counts+1)
             gamma = 0.5/(counts+1)

Sharding: classes (columns of labels) are sharded across the 8 cores.  Each
core streams its own [N, C/8] column block of labels through the PE exactly
once as the moving matmul operand, accumulating S.T = preds.T @ labels in a
single PSUM region over all 128 k-tiles, with per-partition partial counts
accumulated on the vector engine and reduced by one final PE pass against a
ones vector.  Every core computes its C/8 output shard entirely locally —
there is no inter-core collective at all, so nothing serializes behind the
stream.

k-tiles are "virtual": tile q covers sample rows {q + 128*p}.  With that row
order the stationary preds tiles are exactly contiguous column slices of
preds viewed as [128, N*D/128] row-major, so preds loads as a few large
contiguous DMAs instead of thousands of 512-byte descriptors.  Labels are
streamed two k-tiles per DMA: viewing labels as [N/2, 2*C/8], the pair-tile
qq covers k-tiles 2qq/2qq+1 as the strided row slice labels2[qq::64, :],
which makes every DMA descriptor a contiguous 10 KB partition line — twice
the per-descriptor payload of a single-tile load, which matters because the
16 SDMA engines' descriptor processing rate, not HBM, is the binding
resource.

The engines round-robin between DMA queues that have work, so label pairs
alternate between the two fast HWDGE rings (sync / scalar); preds chunks are
interleaved into the same two streams just ahead of first use; center
(host-permuted so one DMA with 5 KB lines yields the class-on-partition
layout) loads once on the otherwise idle gpsimd ring up front.

Counts use two DVE accumulators of pair width: A covers pair-tiles < 56 and
is reduced by ones-matmuls that hide under the tail of the stream; CC covers
the last 8 pairs, so after the last tile only its reduction remains before
the per-class scalars can be formed.
"""

import numpy as np

N, C, D = 16384, 10000, 128
NCORES = 8
CS = C // NCORES   # 1250 classes per core
LR = 0.5
P = 128
KT = N // P        # 128 virtual k-tiles
NPAIR = KT // 2    # 64 pair-tiles
NT3 = (CS + P - 1) // P  # output tiles over the class shard
PCHUNKS = [1024] * 16   # preds load chunks (cols of the [128, N] natural view)
QQSPLIT = 56            # counts accumulator split point (pair index)
assert sum(PCHUNKS) == KT * D


def _chunks(width, step=512):
    out = []
    c0 = 0
    while c0 < width:
        out.append((c0, min(step, width - c0)))
        c0 += step
    return out


def build_program(cs=CS, d=D, kt=KT):
    """Build the SPMD Bass program (identical on every core)."""
    import concourse.bacc as bacc
    import concourse.mybir as mybir
    import concourse.tile as tile
    from concourse.masks import make_identity

    f32 = mybir.dt.float32
    f32r = mybir.dt.float32r
    mult = mybir.AluOpType.mult
    add = mybir.AluOpType.add

    n = kt * P
    nt3 = NT3
    npair = NPAIR
    assert cs * 4 <= 3 * 2048, "S.T PSUM tile must fit in 3 banks"

    nc = bacc.Bacc(
        "TRN2",
        target_bir_lowering=False,
        debug=False,
        num_devices=NCORES,
    )

    # preds in its natural [128, n] row-major view: partition p holds rows
    # [128p, 128p+128); column block [128q, 128q+128) is then exactly the
    # stationary tile for virtual k-tile q (rows 128p+q on partition p).
    # f32r = raw fp32 bits, so plain HWDGE DMAs feed fp32r matmuls directly.
    preds = nc.dram_tensor("preds", [P, kt * d], f32r, kind="ExternalInput").ap()
    # labels pair view: row r = label rows 2r, 2r+1; pair-tile qq is
    # labels2[qq::64, :] (10 KB contiguous per partition line)
    labels2 = nc.dram_tensor(
        "labels", [n // 2, 2 * cs], f32r, kind="ExternalInput"
    ).ap()
    # center arrives host-permuted: element [p, tt*d + j] = center[tt*P + p, j]
    center = nc.dram_tensor("center", [P, nt3 * d], f32, kind="ExternalInput").ap()
    out = nc.dram_tensor("out", [cs, d], f32, kind="ExternalOutput").ap()

    # preds chunk c is needed by k-tile 8c = pair 4c; issue it a couple of
    # pair-tiles early, alternating between the two HWDGE rings.
    trigger_qq = {}
    for cch in range(len(PCHUNKS)):
        trigger_qq.setdefault(max(0, 4 * cch - 2), []).append(cch)

    with tile.TileContext(nc) as tc:
        with tc.tile_pool(name="const", bufs=1) as const_pool:
            identity = const_pool.tile([P, P], f32, name="identity")
            make_identity(nc, identity[:])
            ones_col = const_pool.tile([P, 1], f32, name="ones_col")
            nc.vector.memset(ones_col[:], 1.0)

            # center shard in class-on-partition layout, one 5KB-line DMA
            ctr_sb = const_pool.tile([P, nt3 * d], f32, name="ctr_sb")
            nc.gpsimd.dma_start(out=ctr_sb[:], in_=center[:])

            preds_hi = [
                const_pool.tile([P, pw], f32r, name=f"preds_hi_{cch}")
                for cch, pw in enumerate(PCHUNKS)
            ]
            pstart = [sum(PCHUNKS[:cch]) for cch in range(len(PCHUNKS))]

            # per-partition partial counts (pair width), accumulated on DVE
            acc_a = const_pool.tile([P, 2 * cs], f32, name="acc_a")
            acc_c = const_pool.tile([P, 2 * cs], f32, name="acc_c")

            st_sb = const_pool.tile([d, cs], f32, name="st_sb")
            cnt_row = const_pool.tile([1, cs], f32, name="cnt_row")

            # ---------------- phase 1: stream labels ----------------
            with (
                tc.tile_pool(name="lab", bufs=7) as lab_pool,
                tc.tile_pool(name="psum1", bufs=1, space="PSUM") as psum1,
            ):
                st_psum = psum1.tile([d, cs], f32, name="st_psum", space="PSUM")
                cnt_psum = psum1.tile([1, cs], f32, name="cnt_psum", space="PSUM")
                for qq in range(npair):
                    for cch in trigger_qq.get(qq, []):
                        peng = nc.sync if cch % 2 == 0 else nc.scalar
                        peng.dma_start(
                            out=preds_hi[cch][:],
                            in_=preds[:, pstart[cch]:pstart[cch] + PCHUNKS[cch]],
                        )
                    lab2 = lab_pool.tile(
                        [P, 2 * cs], f32r, name=f"lab_{qq}", tag="lab"
                    )
                    eng = nc.sync if qq % 2 == 0 else nc.scalar
                    eng.dma_start(out=lab2[:], in_=labels2[qq::npair, :])
                    for h in (0, 1):
                        q = 2 * qq + h
                        col = q * d
                        cch = max(
                            i for i in range(len(PCHUNKS)) if pstart[i] <= col
                        )
                        for c0, w in _chunks(cs):
                            nc.tensor.matmul(
                                out=st_psum[:, c0:c0 + w],
                                lhsT=preds_hi[cch][:, col - pstart[cch]:
                                                   col - pstart[cch] + d],
                                rhs=lab2[:, h * cs + c0:h * cs + c0 + w],
                                start=(q == 0),
                                stop=(q == kt - 1),
                            )
                    acc = acc_a if qq < QQSPLIT else acc_c
                    if qq in (0, QQSPLIT):
                        nc.vector.tensor_copy(out=acc[:], in_=lab2[:].bitcast(f32))
                    else:
                        nc.vector.tensor_add(
                            out=acc[:], in0=acc[:], in1=lab2[:].bitcast(f32)
                        )
                    if qq == QQSPLIT + 2:
                        # acc_a is final; its count reduction (both halves into
                        # the same PSUM region) hides under the stream tail
                        for h in (0, 1):
                            for c0, w in _chunks(cs):
                                nc.tensor.matmul(
                                    out=cnt_psum[0:1, c0:c0 + w],
                                    lhsT=ones_col[:],
                                    rhs=acc_a[:, h * cs + c0:h * cs + c0 + w],
                                    start=(h == 0),  # per-region PSUM reset
                                    stop=False,
                                )

                # fold the tail accumulator into the count PSUM accumulation
                for h in (0, 1):
                    for c0, w in _chunks(cs):
                        nc.tensor.matmul(
                            out=cnt_psum[0:1, c0:c0 + w],
                            lhsT=ones_col[:],
                            rhs=acc_c[:, h * cs + c0:h * cs + c0 + w],
                            start=False,
                            stop=(h == 1),  # per-region group close
                        )
                # cnt first: it gates the PE count-transposes below
                nc.scalar.copy(out=cnt_row[:], in_=cnt_psum[:])
                nc.scalar.copy(out=st_sb[:], in_=st_psum[:])

            # ---------------- phase 3: elementwise update, all local -------
            # counts for all nt3 class tiles land as columns of one [P, nt3]
            # PSUM tile, so beta/gamma come from 5 batched DVE ops, and
            # beta*center is one broadcast multiply over the whole shard.
            with (
                tc.tile_pool(name="p3", bufs=2) as p3,
                tc.tile_pool(name="psum3", bufs=1, space="PSUM") as psum3,
            ):
                cnt_all = psum3.tile([P, nt3], f32, name="cnt_all", space="PSUM")
                for tt in range(nt3):
                    w = min(P, cs - tt * P)
                    nc.tensor.transpose(
                        out=cnt_all[0:w, tt:tt + 1],
                        in_=cnt_row[0:1, tt * P:tt * P + w],
                        identity=identity[0:1, 0:1],
                    )
                den = p3.tile([P, nt3], f32, name="den", tag="den", bufs=1)
                nc.vector.tensor_scalar_add(out=den[:], in0=cnt_all[:], scalar1=1.0)
                rec = p3.tile([P, nt3], f32, name="rec", tag="rec", bufs=1)
                nc.vector.reciprocal(out=rec[:], in_=den[:])
                gam = p3.tile([P, nt3], f32, name="gam", tag="gam", bufs=1)
                nc.vector.tensor_scalar_mul(out=gam[:], in0=rec[:], scalar1=0.5)
                bet = p3.tile([P, nt3], f32, name="bet", tag="bet", bufs=1)
                nc.vector.tensor_tensor(
                    out=bet[:], in0=cnt_all[:], in1=rec[:], op=mult
                )
                nc.vector.tensor_scalar(
                    out=bet[:], in0=bet[:],
                    scalar1=-0.5, scalar2=1.0, op0=mult, op1=add,
                )

                # o1 = beta * center for the whole shard in one broadcast op
                o1_all = p3.tile([P, nt3 * d], f32, name="o1_all", tag="o1",
                                 bufs=1)
                nc.vector.tensor_tensor(
                    out=o1_all[:].rearrange("p (t x) -> p t x", x=d),
                    in0=ctr_sb[:].rearrange("p (t x) -> p t x", x=d),
                    in1=bet[:].unsqueeze(2).broadcast_to([P, nt3, d]),
                    op=mult,
                )

                ou_all = p3.tile([P, nt3 * d], f32, name="ou_all", tag="ou",
                                 bufs=1)
                for tt in range(nt3):
                    w = min(P, cs - tt * P)
                    trp = psum3.tile([P, d], f32, name=f"trp_{tt}", tag="trp",
                                     bufs=4, space="PSUM")
                    nc.tensor.transpose(
                        out=trp[0:w, 0:d],
                        in_=st_sb[:, tt * P:tt * P + w],
                        identity=identity[:, 0:d],
                    )
                    nc.vector.scalar_tensor_tensor(
                        out=ou_all[0:w, tt * d:tt * d + d], in0=trp[0:w, 0:d],
                        scalar=gam[0:w, tt:tt + 1],
                        in1=o1_all[0:w, tt * d:tt * d + d], op0=mult, op1=add,
                    )
                # two batched stores: 9 full tiles, then the ragged tail
                nc.sync.dma_start(
                    out=out[0:(nt3 - 1) * P, :]
                        .rearrange("(t p) x -> p t x", p=P),
                    in_=ou_all[:, 0:(nt3 - 1) * d]
                        .rearrange("p (t x) -> p t x", x=d),
                )
                wlast = cs - (nt3 - 1) * P
                nc.scalar.dma_start(
                    out=out[(nt3 - 1) * P:cs, :],
                    in_=ou_all[0:wlast, (nt3 - 1) * d:nt3 * d],
                )

    nc.compile()
    return nc


_PROGRAM = None
LAST_RESULTS = None  # BassKernelResults from the most recent run (for test.py)


def _get_program():
    global _PROGRAM
    if _PROGRAM is None:
        _PROGRAM = build_program()
    return _PROGRAM


def kernel(embeded_preds, labels, center):
    from concourse.bass_utils import run_bass_kernel_spmd

    global LAST_RESULTS
    preds = np.ascontiguousarray(np.asarray(embeded_preds, dtype=np.float32))
    lab = np.ascontiguousarray(np.asarray(labels, dtype=np.float32))
    ctr = np.ascontiguousarray(np.asarray(center, dtype=np.float32))
    assert preds.shape == (N, D) and lab.shape == (N, C) and ctr.shape == (C, D)

    nc = _get_program()
    preds_nat = preds.reshape(P, KT * D)  # free view; bytes unchanged

    def permute_center(cj):
        # [cs, d] -> [P, nt3*d] with [p, tt*d + j] = cj[tt*P + p, j]
        cpad = np.zeros((NT3 * P, D), dtype=np.float32)
        cpad[:cj.shape[0]] = cj
        return np.ascontiguousarray(
            cpad.reshape(NT3, P, D).transpose(1, 0, 2).reshape(P, NT3 * D)
        )

    in_maps = [
        {
            "preds": preds_nat,
            "labels": np.ascontiguousarray(lab[:, j * CS:(j + 1) * CS])
                .reshape(N // 2, 2 * CS),
            "center": permute_center(ctr[j * CS:(j + 1) * CS]),
        }
        for j in range(NCORES)
    ]
    res = run_bass_kernel_spmd(nc, in_maps, core_ids=list(range(NCORES)))
    LAST_RESULTS = res
    return np.concatenate([res.results[j]["out"] for j in range(NCORES)], axis=0)

